# revision 35
# baseline (speedup 1.0000x reference)
"""GemmaAttention (B=2, S=2048, D=2048, H=8, KV=1, HD=256) on 8 trn2 NeuronCores.

Sharding: DP=2 over batch x TP=4 over head-pairs. Core c handles batch c//4 and
heads {2*(c%4), 2*(c%4)+1}. Each core computes its partial o_proj output
(row-parallel Wo); the host sums the 4 partials per batch (the all-reduce is
folded into the host-side unshard).

All PE matmuls run in bf16 (error budget allows it): LDWEIGHTS is 1 cycle/row
instead of fp32r's ~4, and PE power/p-state behaves better. PSUM tiles are
always a full bank, so the kernel is structured around at most 8 live psum
tiles with single-group accumulation chains to keep the PE gap-free (the PE
only reaches its 2.4 GHz p-state after ~3us of continuous execution).

Dataflow per core:
  phase A (projections, full hT resident in SBUF as bf16):
    per s-block of 512 and output pair: psum <- sum_c W[:,c,pair].T @ hT[c,blk]
    RoPE applied in the psum->SBUF drain on DVE, output bf16 (QT/KT).
    V computed directly in [s, dv] layout: psum <- hT_chunk.T @ Wv, drained on
    the scalar engine to bf16 (VN).
  phase C (attention + interleaved o_proj):
    scoresT[k,q] = KT_chunk.T @ QT per head, exp on ACT (scale 1/16 folded),
    causal staircase as a bf16 multiplicative mask, denominators accumulated
    on DVE; outT[dv,q] += V_chunk.T @ expT; normalize with
    reciprocal_approx_fast; out_partial = outTn_chunk.T @ Wo -> DRAM.
"""

import numpy as np
import ml_dtypes

import concourse.bass as bass
import concourse.tile as tile
import concourse.mybir as mybir
from concourse import bacc
from concourse.bass_utils import run_bass_kernel_spmd
from concourse._compat import with_exitstack  # noqa: F401

P = 128
B, S, D = 2, 2048, 2048
H, KV, HD = 8, 1, 256
ROPE_BASE = 10000.0

HEADS_PER_CORE = 2
DQ = HEADS_PER_CORE * HD          # 512 q-dims per core
DCH = D // P                      # 16 contraction chunks
SBLK = 512                        # s-tile for projection rhs / q-tile
NSBLK = S // SBLK                 # 4
NKC = S // P                      # 16 key chunks
NQCH = DQ // P                    # 4 QT partition chunks
NKCH = HD // P                    # 2 KT partition chunks

F32 = mybir.dt.float32
F32R = mybir.dt.float32r
BF16 = mybir.dt.bfloat16
EXP = mybir.ActivationFunctionType.Exp

# exec time of the last traced run (set by run_spmd when tracing)
LAST_EXEC_TIME_NS = None

_BUILD_CACHE = {}


def _build(causal: bool):
    nc = bacc.Bacc()

    hT = nc.declare_dram_parameter("hT", [D, S], BF16, isOutput=False)
    wq = nc.declare_dram_parameter("wq", [D, DQ], BF16, isOutput=False)
    wk = nc.declare_dram_parameter("wk", [D, HD], BF16, isOutput=False)
    wv = nc.declare_dram_parameter("wv", [D, HD], BF16, isOutput=False)
    wo = nc.declare_dram_parameter("wo", [DQ, D], BF16, isOutput=False)
    cosT = nc.declare_dram_parameter("cosT", [HD, S], F32, isOutput=False)
    sinT = nc.declare_dram_parameter("sinT", [HD, S], F32, isOutput=False)
    ones = nc.declare_dram_parameter("ones", [P, P], F32R, isOutput=False)
    onesb = nc.declare_dram_parameter("onesb", [P, P], BF16, isOutput=False)
    identb = nc.declare_dram_parameter("identb", [P, P], BF16, isOutput=False)
    if causal:
        stair = nc.declare_dram_parameter("stair", [P, 2 * SBLK], BF16, isOutput=False)
    else:
        maskT = nc.declare_dram_parameter("amaskT", [S, S], BF16, isOutput=False)
    outp = nc.declare_dram_parameter("out_partial", [S, D], BF16, isOutput=True)

    from contextlib import ExitStack
    from collections import deque
    with tile.TileContext(nc) as tc, ExitStack() as ctx:
        # persistent pools
        pq = ctx.enter_context(tc.tile_pool(name="pq", bufs=1))
        QT = pq.tile([P, NQCH, S], BF16, name="QT")
        KT = pq.tile([P, NKCH, S], BF16, name="KT")
        VN = pq.tile([P, NKC, HD], BF16, name="VN")
        ONES = pq.tile([P, P], F32R, name="ONES")
        ONESB = pq.tile([P, P], BF16, name="ONESB")
        IDENTB = pq.tile([P, P], BF16, name="IDENTB")
        ONEC = ONES[:, 0:1]
        ONERB = ONESB[0:1, :]
        if causal:
            STAIR = pq.tile([P, 2 * SBLK], BF16, name="STAIR")

        # ---- phase A: projections + RoPE --------------------------------
        with tc.tile_pool(name="pht", bufs=1) as pht, \
             tc.tile_pool(name="pw", bufs=1) as pw, \
             tc.tile_pool(name="pcs", bufs=1) as pcs, \
             tc.tile_pool(name="ptmp", bufs=8) as ptmp, \
             tc.tile_pool(name="pjp", bufs=8, space="PSUM") as pp:
            HT = pht.tile([P, DCH, S], BF16, name="HT")
            WQ = pw.tile([P, DCH, DQ], BF16, name="WQ")
            WK = pw.tile([P, DCH, HD], BF16, name="WK")
            WV = pw.tile([P, DCH, HD], BF16, name="WV")
            COS = pcs.tile([P, NKCH, S], F32, name="COS")
            SIN = pcs.tile([P, NKCH, S], F32, name="SIN")

            # DMA order matches consumption. Multi-chunk 3D transfers keep the
            # sync queue from becoming issue-bound (~620ns per descriptor).
            nc.sync.dma_start(out=ONES, in_=ones[:, :])
            nc.sync.dma_start(out=ONESB, in_=onesb[:, :])
            nc.sync.dma_start(out=IDENTB, in_=identb[:, :])
            if causal:
                nc.sync.dma_start(out=STAIR, in_=stair[:, :])

            def chunked3d(out_tile, dram, c0, c1, csl):
                # out_tile[:, c0:c1, csl] <- dram rows [c0*P, c1*P), cols csl
                nc.sync.dma_start(
                    out=out_tile[:, c0:c1, csl],
                    in_=dram.rearrange("(c p) n -> p c n", p=P)[:, c0:c1, csl])

            full = slice(0, None)
            # sb0: WQ and hT slabs interleaved, small first slab for fast start
            for c0, c1 in ((0, 2), (2, 6), (6, 11), (11, 16)):
                chunked3d(WQ, wq, c0, c1, full)
                chunked3d(HT, hT, c0, c1, slice(0, SBLK))
            chunked3d(COS, cosT, 0, NKCH, slice(0, SBLK))
            chunked3d(SIN, sinT, 0, NKCH, slice(0, SBLK))
            chunked3d(WK, wk, 0, DCH, full)
            chunked3d(WV, wv, 0, DCH, full)
            for sb in range(1, NSBLK):
                ssl = slice(sb * SBLK, (sb + 1) * SBLK)
                chunked3d(HT, hT, 0, DCH, ssl)
                chunked3d(COS, cosT, 0, NKCH, ssl)
                chunked3d(SIN, sinT, 0, NKCH, ssl)

            def rope_pair(p0, p1, out0, out1):
                # out0 = p0*cos0 - p1*sin0 ; out1 = p1*cos1 + p0*sin1
                c0 = COS[:, 0, ssl]; c1 = COS[:, 1, ssl]
                s0 = SIN[:, 0, ssl]; s1 = SIN[:, 1, ssl]
                t1 = ptmp.tile([P, SBLK], F32, name="t")
                t2 = ptmp.tile([P, SBLK], F32, name="t")
                nc.vector.tensor_mul(t1, p0, c0)
                nc.vector.tensor_mul(t2, p1, s0)
                nc.vector.tensor_sub(out0, t1, t2)
                t3 = ptmp.tile([P, SBLK], F32, name="t")
                t4 = ptmp.tile([P, SBLK], F32, name="t")
                nc.vector.tensor_mul(t3, p1, c1)
                nc.vector.tensor_mul(t4, p0, s1)
                nc.vector.tensor_add(out1, t3, t4)

            for sb in range(NSBLK):
                ssl = slice(sb * SBLK, (sb + 1) * SBLK)
                # Q pairs (per head), then K pair: 16-matmul psum groups
                for h in range(HEADS_PER_CORE):
                    p0 = pp.tile([P, SBLK], F32, name="pp")
                    p1 = pp.tile([P, SBLK], F32, name="pp")
                    for c in range(DCH):
                        nc.tensor.matmul(p0, lhsT=WQ[:, c, 2 * h * P:(2 * h + 1) * P],
                                         rhs=HT[:, c, ssl],
                                         start=(c == 0), stop=(c == DCH - 1))
                        nc.tensor.matmul(p1, lhsT=WQ[:, c, (2 * h + 1) * P:(2 * h + 2) * P],
                                         rhs=HT[:, c, ssl],
                                         start=(c == 0), stop=(c == DCH - 1))
                    rope_pair(p0, p1, QT[:, 2 * h, ssl], QT[:, 2 * h + 1, ssl])
                p0 = pp.tile([P, SBLK], F32, name="pp")
                p1 = pp.tile([P, SBLK], F32, name="pp")
                for c in range(DCH):
                    nc.tensor.matmul(p0, lhsT=WK[:, c, 0:P], rhs=HT[:, c, ssl],
                                     start=(c == 0), stop=(c == DCH - 1))
                    nc.tensor.matmul(p1, lhsT=WK[:, c, P:2 * P], rhs=HT[:, c, ssl],
                                     start=(c == 0), stop=(c == DCH - 1))
                rope_pair(p0, p1, KT[:, 0, ssl], KT[:, 1, ssl])
                # V directly in [s, dv] layout (lhsT = hT chunk slice)
                for si in range(SBLK // P):
                    sv = pp.tile([P, SBLK], F32, name="pp")[:, 0:HD]
                    soff = sb * SBLK + si * P
                    for c in range(DCH):
                        nc.tensor.matmul(sv, lhsT=HT[:, c, soff:soff + P],
                                         rhs=WV[:, c, :],
                                         start=(c == 0), stop=(c == DCH - 1))
                    nc.scalar.copy(VN[:, sb * (SBLK // P) + si, :], sv)

        # ---- late persistent: o_proj weights + normalized outT ----------
        patt = ctx.enter_context(tc.tile_pool(name="patt", bufs=1))
        WO = patt.tile([P, NQCH, D], BF16, name="WO")
        nc.sync.dma_start(out=WO, in_=wo.rearrange("(c p) n -> p c n", p=P))
        OUTN = patt.tile([P, NQCH, S], BF16, name="OUTN")

        # ---- phase C: attention (both heads interleaved) + o_proj -------
        # One kc loop drives both heads: the PE gets 2x the independent work
        # between a score tile and its AV consumer, so the exp->mask chain has
        # ~2us of slack. Norm and o_proj work from the previous q-block is
        # injected between kc steps as small jobs (never bursts), keeping the
        # ACT/DVE queues shallow for the latency-critical exp path.
        with tc.tile_pool(name="pexp", bufs=10) as pexp, \
             tc.tile_pool(name="pacc", bufs=12) as pacc, \
             tc.tile_pool(name="pou", bufs=8) as pou, \
             tc.tile_pool(name="pmisc", bufs=8) as pmisc, \
             tc.tile_pool(name="pmask", bufs=2) as pmask, \
             tc.tile_pool(name="pfin", bufs=4) as pfin, \
             tc.tile_pool(name="ps_s", bufs=3, space="PSUM") as ps_s, \
             tc.tile_pool(name="ps_o", bufs=4, space="PSUM") as ps_o, \
             tc.tile_pool(name="ps_f", bufs=1, space="PSUM") as ps_f:

            def make_norm_jobs(ous, accs, qb):
                qsl = slice(qb * SBLK, (qb + 1) * SBLK)
                rsbbs = {}

                def ja(h):
                    def f():
                        pssum = ps_s.tile([P, SBLK], F32, name="ps")
                        nc.tensor.matmul(pssum[0:1, :], lhsT=ONEC, rhs=accs[h])
                        rsb = pmisc.tile([1, SBLK], F32, name="rsb")
                        rsbb = pmisc.tile([1, SBLK], BF16, name="rsbb")
                        with nc.allow_low_precision("approx softmax recip"):
                            nc.vector.reciprocal_approx_fast(out=rsb,
                                                             in_=pssum[0:1, :])
                            nc.vector.tensor_copy(rsbb, rsb)
                        rsbbs[h] = rsbb
                    return f

                def jb(h):
                    def f():
                        psb = ps_s.tile([P, SBLK], F32, name="ps")
                        nc.tensor.matmul(psb, lhsT=ONERB, rhs=rsbbs[h])
                        rbc = pmisc.tile([P, SBLK], F32R, name="rbc")
                        nc.scalar.copy(rbc, psb)
                        for dvc in range(2):
                            nc.vector.tensor_mul(OUTN[:, 2 * h + dvc, qsl],
                                                 ous[h][dvc], rbc)
                    return f

                return [ja(0), ja(1), jb(0), jb(1)]

            def make_oproj_jobs(qb):
                # one job per (st, nb) output tile: 4 matmuls + drain + DMA
                jobs = []
                for st in range(4 * qb, 4 * qb + 4):
                    for nb in range(NSBLK):
                        def f(st=st, nb=nb):
                            stsl = slice(st * P, (st + 1) * P)
                            psf = ps_f.tile([P, SBLK], F32, name="pf")
                            for dvc in range(NQCH):
                                nc.tensor.matmul(
                                    psf, lhsT=OUTN[:, dvc, stsl],
                                    rhs=WO[:, dvc, nb * SBLK:(nb + 1) * SBLK],
                                    start=(dvc == 0), stop=(dvc == NQCH - 1))
                            # bf16 drain (halves output DMA); mostly on DVE,
                            # every 4th on ACT (exp keeps ACT ~80% busy)
                            fsb = pfin.tile([P, SBLK], BF16, name="fsb")
                            if nb % 4 == 0:
                                nc.scalar.copy(fsb, psf)
                            else:
                                with nc.allow_low_precision("bf16 o_proj out"):
                                    nc.vector.tensor_copy(fsb, psf)
                            nc.sync.dma_start(
                                out=outp[stsl, nb * SBLK:(nb + 1) * SBLK],
                                in_=fsb)
                        jobs.append(f)
                return jobs

            backlog = deque()
            for qb in range(NSBLK):
                qsl = slice(qb * SBLK, (qb + 1) * SBLK)
                klim = 4 * (qb + 1) if causal else NKC
                MT = None
                if not causal:
                    MT = pmask.tile([P, NKC, SBLK], BF16, name="mt")
                    nc.sync.dma_start(
                        out=MT,
                        in_=maskT.rearrange("(c p) n -> p c n", p=P)[:, :, qsl])

                pso = {h: [ps_o.tile([P, SBLK], F32, name="po")
                           for _ in range(2)] for h in range(2)}
                exs = {}
                parts = {0: [], 1: []}

                # pairwise denominator tree: leaf merges on gpsimd (idle),
                # higher ranks on DVE; chain depth after the last ex is ~2
                def acc_push(h, node):
                    rank = 0
                    while parts[h] and parts[h][-1][0] == rank:
                        _, prev = parts[h].pop()
                        eng = nc.gpsimd if rank == 0 else nc.vector
                        t = pacc.tile([P, SBLK], F32R, name="acc")
                        with nc.allow_low_precision("softmax denom partial"):
                            eng.tensor_add(t, prev, node)
                        node = t
                        rank += 1
                    parts[h].append((rank, node))

                def acc_flush(h):
                    _, node = parts[h].pop()
                    while parts[h]:
                        _, prev = parts[h].pop()
                        t = pacc.tile([P, SBLK], F32R, name="acc")
                        nc.vector.tensor_add(t, prev, node)
                        node = t
                    return node

                def emit_scores(h, kc):
                    # causal/external mask folded into the PE as a third
                    # matmul (identity lhsT x additive -1e4 staircase): no
                    # DVE op in the exp->AV latency chain, exp(-625) == 0
                    mask_rhs = None
                    if causal and kc >= 4 * qb:
                        delta = 128 * kc - 512 * qb
                        mask_rhs = STAIR[:, 512 - delta:1024 - delta]
                    elif not causal:
                        mask_rhs = MT[:, kc, :]
                    pss = ps_s.tile([P, SBLK], F32, name="ps")
                    for c in range(NKCH):
                        nc.tensor.matmul(pss,
                                         lhsT=KT[:, c, kc * P:(kc + 1) * P],
                                         rhs=QT[:, 2 * h + c, qsl],
                                         start=(c == 0),
                                         stop=(c == NKCH - 1 and mask_rhs is None))
                    if mask_rhs is not None:
                        nc.tensor.matmul(pss, lhsT=IDENTB, rhs=mask_rhs,
                                         start=False, stop=True)
                    ex = pexp.tile([P, SBLK], BF16, name="ex")
                    nc.scalar.activation(ex, pss, EXP, scale=1.0 / 16.0)
                    acc_push(h, ex)
                    exs[(h, kc)] = ex

                def emit_av(h, kc):
                    ex = exs.pop((h, kc))
                    for dvc in range(2):
                        nc.tensor.matmul(pso[h][dvc],
                                         lhsT=VN[:, kc, dvc * P:(dvc + 1) * P],
                                         rhs=ex, start=(kc == 0),
                                         stop=(kc == klim - 1))

                inject = list(backlog)
                backlog = deque()
                ninj, ptr = len(inject), 0
                npts = max(1, klim - 2)
                for kc in range(klim):
                    emit_scores(0, kc)
                    emit_scores(1, kc)
                    if kc >= 2:
                        # spread backlog jobs (prev qb's norm + o_proj) evenly
                        want = ninj * (kc - 1) // npts
                        while ptr < want:
                            inject[ptr]()
                            ptr += 1
                        emit_av(0, kc - 2)
                        emit_av(1, kc - 2)
                while ptr < ninj:
                    inject[ptr]()
                    ptr += 1
                for kc in range(max(0, klim - 2), klim):
                    emit_av(0, kc)
                    emit_av(1, kc)
                accs = {h: acc_flush(h) for h in range(2)}
                ous = {}
                for h in range(2):
                    ous[h] = [pou.tile([P, SBLK], BF16, name="ou")
                              for _ in range(2)]
                    for dvc in range(2):
                        nc.vector.tensor_copy(ous[h][dvc], pso[h][dvc])
                backlog.extend(make_norm_jobs(ous, accs, qb))
                backlog.extend(make_oproj_jobs(qb))
            while backlog:
                backlog.popleft()()

    nc.finalize()
    return nc


def _get_nc(causal: bool):
    key = bool(causal)
    if key not in _BUILD_CACHE:
        _BUILD_CACHE[key] = _build(causal)
    return _BUILD_CACHE[key]


def _rope_tables(position_ids_b):
    # cosT/sinT: [HD, S] fp32, transposed layout for the [d, s] dataflow
    pos = np.asarray(position_ids_b, dtype=np.float64)
    inv = 1.0 / (ROPE_BASE ** (np.arange(0, HD, 2, dtype=np.float64) / HD))
    f = pos[:, None] * inv[None, :]            # [S, HD/2]
    emb = np.concatenate([f, f], axis=1)       # [S, HD]
    cosT = np.ascontiguousarray(np.cos(emb).T.astype(np.float32))
    sinT = np.ascontiguousarray(np.sin(emb).T.astype(np.float32))
    return cosT, sinT


def _is_causal(attention_mask):
    m = np.asarray(attention_mask)
    if m.shape != (B, 1, S, S):
        return False
    tri = np.tril(np.ones((S, S), dtype=bool))
    canon = np.where(tri, np.float32(0.0), np.float32(-1e9))
    return all(np.array_equal(m[b, 0], canon) for b in range(B))


_ONES_NP = np.ones((P, P), dtype=np.float32)
_ONES_BF = np.ones((P, P), dtype=ml_dtypes.bfloat16)
_IDENT_BF = np.eye(P, dtype=np.float32).astype(ml_dtypes.bfloat16)


NEG_MASK = -1.0e4  # additive logit mask; exp(-1e4/16) == 0 exactly in fp32


def _stair():
    # additive staircase: 0 where (j - 512) >= p (keep), -1e4 above diagonal
    j = np.arange(2 * SBLK)[None, :] - SBLK
    p = np.arange(P)[:, None]
    return np.where(j >= p, 0.0, NEG_MASK).astype(ml_dtypes.bfloat16)


def _bf(x):
    return np.ascontiguousarray(np.asarray(x, dtype=np.float32).astype(ml_dtypes.bfloat16))


def kernel(hidden_state, attention_mask, position_ids, Wq, Wk, Wv, Wo,
           _trace=False, _tmpdir=None):
    global LAST_EXEC_TIME_NS
    hidden_state = np.asarray(hidden_state, dtype=np.float32)

    causal = _is_causal(attention_mask)
    nc = _get_nc(causal)

    stair = _stair() if causal else None
    wk_bf = _bf(Wk)
    wv_bf = _bf(Wv)
    per_batch = {}
    for b in range(B):
        hTb = _bf(hidden_state[b].T)                           # [D, S] bf16
        cosT, sinT = _rope_tables(position_ids[b])
        mb = None
        if not causal:
            # additive mask in pre-scale logit units: exp((pss + m)/16)
            m16 = 16.0 * np.asarray(attention_mask, dtype=np.float64)[b, 0].T
            mb = np.ascontiguousarray(
                np.maximum(m16, NEG_MASK).astype(ml_dtypes.bfloat16))
        per_batch[b] = (hTb, cosT, sinT, mb)

    in_maps = []
    for core in range(8):
        b = core // 4
        hp = core % 4
        hTb, cosT, sinT, mb = per_batch[b]
        im = {
            "hT": hTb,
            "ones": _ONES_NP,
            "onesb": _ONES_BF,
            "identb": _IDENT_BF,
            "wq": _bf(Wq[:, hp * DQ:(hp + 1) * DQ]),
            "wk": wk_bf,
            "wv": wv_bf,
            "wo": _bf(Wo[hp * DQ:(hp + 1) * DQ, :]),
            "cosT": cosT,
            "sinT": sinT,
        }
        if causal:
            im["stair"] = stair
        else:
            im["amaskT"] = mb
        in_maps.append(im)

    res = run_bass_kernel_spmd(nc, in_maps, core_ids=list(range(8)),
                               trace=_trace, tmpdir=_tmpdir)
    LAST_EXEC_TIME_NS = res.exec_time_ns

    out = np.empty((B, S, D), dtype=np.float32)
    for b in range(B):
        acc = res.results[4 * b]["out_partial"].astype(np.float32).copy()
        for hp in range(1, 4):
            acc += res.results[4 * b + hp]["out_partial"]
        out[b] = acc
    return out


# revision 37
# speedup vs baseline: 1.0178x; 1.0178x over previous
"""GemmaAttention (B=2, S=2048, D=2048, H=8, KV=1, HD=256) on 8 trn2 NeuronCores.

Sharding: DP=2 over batch x TP=4 over head-pairs. Core c handles batch c//4 and
heads {2*(c%4), 2*(c%4)+1}. Each core computes its partial o_proj output
(row-parallel Wo); the host sums the 4 partials per batch (the all-reduce is
folded into the host-side unshard).

All PE matmuls run in bf16 (error budget allows it): LDWEIGHTS is 1 cycle/row
instead of fp32r's ~4, and PE power/p-state behaves better. PSUM tiles are
always a full bank, so the kernel is structured around at most 8 live psum
tiles with single-group accumulation chains to keep the PE gap-free (the PE
only reaches its 2.4 GHz p-state after ~3us of continuous execution).

Dataflow per core:
  phase A (projections, full hT resident in SBUF as bf16):
    per s-block of 512 and output pair: psum <- sum_c W[:,c,pair].T @ hT[c,blk]
    RoPE applied in the psum->SBUF drain on DVE, output bf16 (QT/KT).
    V computed directly in [s, dv] layout: psum <- hT_chunk.T @ Wv, drained on
    the scalar engine to bf16 (VN).
  phase C (attention + interleaved o_proj):
    scoresT[k,q] = KT_chunk.T @ QT per head, exp on ACT (scale 1/16 folded),
    causal staircase as a bf16 multiplicative mask, denominators accumulated
    on DVE; outT[dv,q] += V_chunk.T @ expT; normalize with
    reciprocal_approx_fast; out_partial = outTn_chunk.T @ Wo -> DRAM.
"""

import numpy as np
import ml_dtypes

import concourse.bass as bass
import concourse.tile as tile
import concourse.mybir as mybir
from concourse import bacc
from concourse.bass_utils import run_bass_kernel_spmd
from concourse._compat import with_exitstack  # noqa: F401

P = 128
B, S, D = 2, 2048, 2048
H, KV, HD = 8, 1, 256
ROPE_BASE = 10000.0

HEADS_PER_CORE = 2
DQ = HEADS_PER_CORE * HD          # 512 q-dims per core
DCH = D // P                      # 16 contraction chunks
SBLK = 512                        # s-tile for projection rhs / q-tile
NSBLK = S // SBLK                 # 4
NKC = S // P                      # 16 key chunks
NQCH = DQ // P                    # 4 QT partition chunks
NKCH = HD // P                    # 2 KT partition chunks

F32 = mybir.dt.float32
F32R = mybir.dt.float32r
BF16 = mybir.dt.bfloat16
EXP = mybir.ActivationFunctionType.Exp

# exec time of the last traced run (set by run_spmd when tracing)
LAST_EXEC_TIME_NS = None

_BUILD_CACHE = {}


def _build(causal: bool):
    nc = bacc.Bacc()

    hT = nc.declare_dram_parameter("hT", [D, S], BF16, isOutput=False)
    wq = nc.declare_dram_parameter("wq", [D, DQ], BF16, isOutput=False)
    wk = nc.declare_dram_parameter("wk", [D, HD], BF16, isOutput=False)
    wv = nc.declare_dram_parameter("wv", [D, HD], BF16, isOutput=False)
    wo = nc.declare_dram_parameter("wo", [DQ, D], BF16, isOutput=False)
    cosT = nc.declare_dram_parameter("cosT", [HD, S], F32, isOutput=False)
    sinT = nc.declare_dram_parameter("sinT", [HD, S], F32, isOutput=False)
    ones = nc.declare_dram_parameter("ones", [P, P], F32R, isOutput=False)
    onesb = nc.declare_dram_parameter("onesb", [P, P], BF16, isOutput=False)
    identb = nc.declare_dram_parameter("identb", [P, P], BF16, isOutput=False)
    if causal:
        stair = nc.declare_dram_parameter("stair", [P, 2 * SBLK], BF16, isOutput=False)
    else:
        maskT = nc.declare_dram_parameter("amaskT", [S, S], BF16, isOutput=False)
    outp = nc.declare_dram_parameter("out_partial", [S, D], BF16, isOutput=True)

    from contextlib import ExitStack
    from collections import deque
    with tile.TileContext(nc) as tc, ExitStack() as ctx:
        # persistent pools
        pq = ctx.enter_context(tc.tile_pool(name="pq", bufs=1))
        QT = pq.tile([P, NQCH, S], BF16, name="QT")
        KT = pq.tile([P, NKCH, S], BF16, name="KT")
        VN = pq.tile([P, NKC, HD], BF16, name="VN")
        ONES = pq.tile([P, P], F32R, name="ONES")
        ONESB = pq.tile([P, P], BF16, name="ONESB")
        IDENTB = pq.tile([P, P], BF16, name="IDENTB")
        ONEC = ONES[:, 0:1]
        ONERB = ONESB[0:1, :]
        if causal:
            STAIR = pq.tile([P, 2 * SBLK], BF16, name="STAIR")

        # ---- phase A: projections + RoPE --------------------------------
        with tc.tile_pool(name="pht", bufs=1) as pht, \
             tc.tile_pool(name="pw", bufs=1) as pw, \
             tc.tile_pool(name="pcs", bufs=1) as pcs, \
             tc.tile_pool(name="ptmp", bufs=8) as ptmp, \
             tc.tile_pool(name="pjp", bufs=8, space="PSUM") as pp:
            HT = pht.tile([P, DCH, S], BF16, name="HT")
            WQ = pw.tile([P, DCH, DQ], BF16, name="WQ")
            WK = pw.tile([P, DCH, HD], BF16, name="WK")
            WV = pw.tile([P, DCH, HD], BF16, name="WV")
            COS = pcs.tile([P, NKCH, S], F32, name="COS")
            SIN = pcs.tile([P, NKCH, S], F32, name="SIN")

            # DMA order matches consumption. Multi-chunk 3D transfers keep the
            # sync queue from becoming issue-bound (~620ns per descriptor).
            nc.sync.dma_start(out=ONES, in_=ones[:, :])
            nc.sync.dma_start(out=ONESB, in_=onesb[:, :])
            nc.sync.dma_start(out=IDENTB, in_=identb[:, :])
            if causal:
                nc.sync.dma_start(out=STAIR, in_=stair[:, :])

            def chunked3d(out_tile, dram, c0, c1, csl):
                # out_tile[:, c0:c1, csl] <- dram rows [c0*P, c1*P), cols csl
                nc.sync.dma_start(
                    out=out_tile[:, c0:c1, csl],
                    in_=dram.rearrange("(c p) n -> p c n", p=P)[:, c0:c1, csl])

            full = slice(0, None)
            # sb0: WQ and hT slabs interleaved, small first slab for fast start
            for c0, c1 in ((0, 2), (2, 6), (6, 11), (11, 16)):
                chunked3d(WQ, wq, c0, c1, full)
                chunked3d(HT, hT, c0, c1, slice(0, SBLK))
            chunked3d(COS, cosT, 0, NKCH, slice(0, SBLK))
            chunked3d(SIN, sinT, 0, NKCH, slice(0, SBLK))
            chunked3d(WK, wk, 0, DCH, full)
            chunked3d(WV, wv, 0, DCH, full)
            for sb in range(1, NSBLK):
                ssl = slice(sb * SBLK, (sb + 1) * SBLK)
                chunked3d(HT, hT, 0, DCH, ssl)
                chunked3d(COS, cosT, 0, NKCH, ssl)
                chunked3d(SIN, sinT, 0, NKCH, ssl)

            def rope_pair(p0, p1, out0, out1):
                # out0 = p0*cos0 - p1*sin0 ; out1 = p1*cos1 + p0*sin1
                c0 = COS[:, 0, ssl]; c1 = COS[:, 1, ssl]
                s0 = SIN[:, 0, ssl]; s1 = SIN[:, 1, ssl]
                t1 = ptmp.tile([P, SBLK], F32, name="t")
                t2 = ptmp.tile([P, SBLK], F32, name="t")
                nc.vector.tensor_mul(t1, p0, c0)
                nc.vector.tensor_mul(t2, p1, s0)
                nc.vector.tensor_sub(out0, t1, t2)
                t3 = ptmp.tile([P, SBLK], F32, name="t")
                t4 = ptmp.tile([P, SBLK], F32, name="t")
                nc.vector.tensor_mul(t3, p1, c1)
                nc.vector.tensor_mul(t4, p0, s1)
                nc.vector.tensor_add(out1, t3, t4)

            for sb in range(NSBLK):
                ssl = slice(sb * SBLK, (sb + 1) * SBLK)
                # Q pairs (per head), then K pair: 16-matmul psum groups
                for h in range(HEADS_PER_CORE):
                    p0 = pp.tile([P, SBLK], F32, name="pp")
                    p1 = pp.tile([P, SBLK], F32, name="pp")
                    for c in range(DCH):
                        nc.tensor.matmul(p0, lhsT=WQ[:, c, 2 * h * P:(2 * h + 1) * P],
                                         rhs=HT[:, c, ssl],
                                         start=(c == 0), stop=(c == DCH - 1))
                        nc.tensor.matmul(p1, lhsT=WQ[:, c, (2 * h + 1) * P:(2 * h + 2) * P],
                                         rhs=HT[:, c, ssl],
                                         start=(c == 0), stop=(c == DCH - 1))
                    rope_pair(p0, p1, QT[:, 2 * h, ssl], QT[:, 2 * h + 1, ssl])
                p0 = pp.tile([P, SBLK], F32, name="pp")
                p1 = pp.tile([P, SBLK], F32, name="pp")
                for c in range(DCH):
                    nc.tensor.matmul(p0, lhsT=WK[:, c, 0:P], rhs=HT[:, c, ssl],
                                     start=(c == 0), stop=(c == DCH - 1))
                    nc.tensor.matmul(p1, lhsT=WK[:, c, P:2 * P], rhs=HT[:, c, ssl],
                                     start=(c == 0), stop=(c == DCH - 1))
                rope_pair(p0, p1, KT[:, 0, ssl], KT[:, 1, ssl])
                # V directly in [s, dv] layout (lhsT = hT chunk slice)
                for si in range(SBLK // P):
                    sv = pp.tile([P, SBLK], F32, name="pp")[:, 0:HD]
                    soff = sb * SBLK + si * P
                    for c in range(DCH):
                        nc.tensor.matmul(sv, lhsT=HT[:, c, soff:soff + P],
                                         rhs=WV[:, c, :],
                                         start=(c == 0), stop=(c == DCH - 1))
                    nc.scalar.copy(VN[:, sb * (SBLK // P) + si, :], sv)

        # ---- late persistent: o_proj weights + normalized outT ----------
        patt = ctx.enter_context(tc.tile_pool(name="patt", bufs=1))
        WO = patt.tile([P, NQCH, D], BF16, name="WO")
        nc.sync.dma_start(out=WO, in_=wo.rearrange("(c p) n -> p c n", p=P))
        OUTN = patt.tile([P, NQCH, S], BF16, name="OUTN")

        # ---- phase C: attention (both heads interleaved) + o_proj -------
        # One kc loop drives both heads: the PE gets 2x the independent work
        # between a score tile and its AV consumer, so the exp->mask chain has
        # ~2us of slack. Norm and o_proj work from the previous q-block is
        # injected between kc steps as small jobs (never bursts), keeping the
        # ACT/DVE queues shallow for the latency-critical exp path.
        with tc.tile_pool(name="pexp", bufs=10) as pexp, \
             tc.tile_pool(name="pacc", bufs=12) as pacc, \
             tc.tile_pool(name="pou", bufs=8) as pou, \
             tc.tile_pool(name="pmisc", bufs=8) as pmisc, \
             tc.tile_pool(name="pmask", bufs=2) as pmask, \
             tc.tile_pool(name="pfin", bufs=4) as pfin, \
             tc.tile_pool(name="ps_s", bufs=3, space="PSUM") as ps_s, \
             tc.tile_pool(name="ps_o", bufs=4, space="PSUM") as ps_o, \
             tc.tile_pool(name="ps_f", bufs=1, space="PSUM") as ps_f:

            def make_norm_jobs(ous, accs, qb):
                qsl = slice(qb * SBLK, (qb + 1) * SBLK)
                rsbbs = {}

                def ja(h):
                    def f():
                        pssum = ps_s.tile([P, SBLK], F32, name="ps")
                        nc.tensor.matmul(pssum[0:1, :], lhsT=ONEC, rhs=accs[h])
                        rsb = pmisc.tile([1, SBLK], F32, name="rsb")
                        rsbb = pmisc.tile([1, SBLK], BF16, name="rsbb")
                        with nc.allow_low_precision("approx softmax recip"):
                            nc.vector.reciprocal_approx_fast(out=rsb,
                                                             in_=pssum[0:1, :])
                            nc.vector.tensor_copy(rsbb, rsb)
                        rsbbs[h] = rsbb
                    return f

                def jb(h):
                    def f():
                        psb = ps_s.tile([P, SBLK], F32, name="ps")
                        nc.tensor.matmul(psb, lhsT=ONERB, rhs=rsbbs[h])
                        rbc = pmisc.tile([P, SBLK], F32R, name="rbc")
                        nc.scalar.copy(rbc, psb)
                        for dvc in range(2):
                            nc.vector.tensor_mul(OUTN[:, 2 * h + dvc, qsl],
                                                 ous[h][dvc], rbc)
                    return f

                return [ja(0), ja(1), jb(0), jb(1)]

            def make_oproj_jobs(qb):
                # one job per (st, nb) output tile: 4 matmuls + drain + DMA
                jobs = []
                for st in range(4 * qb, 4 * qb + 4):
                    for nb in range(NSBLK):
                        def f(st=st, nb=nb):
                            stsl = slice(st * P, (st + 1) * P)
                            psf = ps_f.tile([P, SBLK], F32, name="pf")
                            for dvc in range(NQCH):
                                nc.tensor.matmul(
                                    psf, lhsT=OUTN[:, dvc, stsl],
                                    rhs=WO[:, dvc, nb * SBLK:(nb + 1) * SBLK],
                                    start=(dvc == 0), stop=(dvc == NQCH - 1))
                            # bf16 drain (halves output DMA); mostly on DVE,
                            # every 4th on ACT (exp keeps ACT ~80% busy)
                            fsb = pfin.tile([P, SBLK], BF16, name="fsb")
                            if nb % 4 == 0:
                                nc.scalar.copy(fsb, psf)
                            else:
                                with nc.allow_low_precision("bf16 o_proj out"):
                                    nc.vector.tensor_copy(fsb, psf)
                            nc.sync.dma_start(
                                out=outp[stsl, nb * SBLK:(nb + 1) * SBLK],
                                in_=fsb)
                        jobs.append(f)
                return jobs

            backlog = deque()
            for qb in range(NSBLK):
                qsl = slice(qb * SBLK, (qb + 1) * SBLK)
                klim = 4 * (qb + 1) if causal else NKC
                MT = None
                if not causal:
                    MT = pmask.tile([P, NKC, SBLK], BF16, name="mt")
                    nc.sync.dma_start(
                        out=MT,
                        in_=maskT.rearrange("(c p) n -> p c n", p=P)[:, :, qsl])

                pso = {h: [ps_o.tile([P, SBLK], F32, name="po")
                           for _ in range(2)] for h in range(2)}
                exs = {}
                parts = {0: [], 1: []}

                # pairwise denominator tree: leaf merges on gpsimd (idle),
                # higher ranks on DVE; chain depth after the last ex is ~2
                def acc_push(h, node):
                    rank = 0
                    while parts[h] and parts[h][-1][0] == rank:
                        _, prev = parts[h].pop()
                        eng = nc.gpsimd if rank == 0 else nc.vector
                        t = pacc.tile([P, SBLK], F32R, name="acc")
                        with nc.allow_low_precision("softmax denom partial"):
                            eng.tensor_add(t, prev, node)
                        node = t
                        rank += 1
                    parts[h].append((rank, node))

                def acc_flush(h):
                    _, node = parts[h].pop()
                    while parts[h]:
                        _, prev = parts[h].pop()
                        t = pacc.tile([P, SBLK], F32R, name="acc")
                        nc.vector.tensor_add(t, prev, node)
                        node = t
                    return node

                def emit_scores(h, kc):
                    # causal/external mask folded into the PE as a third
                    # matmul (identity lhsT x additive -1e4 staircase): no
                    # DVE op in the exp->AV latency chain, exp(-625) == 0
                    mask_rhs = None
                    if causal and kc >= 4 * qb:
                        delta = 128 * kc - 512 * qb
                        mask_rhs = STAIR[:, 512 - delta:1024 - delta]
                    elif not causal:
                        mask_rhs = MT[:, kc, :]
                    pss = ps_s.tile([P, SBLK], F32, name="ps")
                    for c in range(NKCH):
                        nc.tensor.matmul(pss,
                                         lhsT=KT[:, c, kc * P:(kc + 1) * P],
                                         rhs=QT[:, 2 * h + c, qsl],
                                         start=(c == 0),
                                         stop=(c == NKCH - 1 and mask_rhs is None))
                    if mask_rhs is not None:
                        nc.tensor.matmul(pss, lhsT=IDENTB, rhs=mask_rhs,
                                         start=False, stop=True)
                    ex = pexp.tile([P, SBLK], BF16, name="ex")
                    nc.scalar.activation(ex, pss, EXP, scale=1.0 / 16.0)
                    acc_push(h, ex)
                    exs[(h, kc)] = ex

                def emit_av(h, kc):
                    ex = exs.pop((h, kc))
                    for dvc in range(2):
                        nc.tensor.matmul(pso[h][dvc],
                                         lhsT=VN[:, kc, dvc * P:(dvc + 1) * P],
                                         rhs=ex, start=(kc == 0),
                                         stop=(kc == klim - 1))

                bl = list(backlog)
                norm_jobs, st_jobs = (bl[:4], bl[4:]) if len(bl) >= 4 else ([], bl)
                backlog = deque()
                nst, ptr = len(st_jobs), 0
                npts = max(1, klim - 5)
                for kc in range(klim):
                    emit_scores(0, kc)
                    emit_scores(1, kc)
                    # norm jobs pinned early: ja pair at kc=2, jb pair at kc=4
                    # (the jb matmul then has ~2 steps of slack on the recip);
                    # o_proj jobs spread over the remaining steps
                    if kc == 2 and norm_jobs:
                        norm_jobs[0](); norm_jobs[1]()
                    if kc == 4 and norm_jobs:
                        norm_jobs[2](); norm_jobs[3]()
                    if kc >= 5:
                        want = nst * (kc - 4) // npts
                        while ptr < want:
                            st_jobs[ptr]()
                            ptr += 1
                    if kc >= 2:
                        emit_av(0, kc - 2)
                        emit_av(1, kc - 2)
                while ptr < nst:
                    st_jobs[ptr]()
                    ptr += 1
                for kc in range(max(0, klim - 2), klim):
                    emit_av(0, kc)
                    emit_av(1, kc)
                accs = {h: acc_flush(h) for h in range(2)}
                ous = {}
                for h in range(2):
                    ous[h] = [pou.tile([P, SBLK], BF16, name="ou")
                              for _ in range(2)]
                    for dvc in range(2):
                        # drain AV psums on ACT: frees the pso banks fast and
                        # keeps the DVE queue short for the upcoming recip
                        nc.scalar.copy(ous[h][dvc], pso[h][dvc])
                backlog.extend(make_norm_jobs(ous, accs, qb))
                backlog.extend(make_oproj_jobs(qb))
            while backlog:
                backlog.popleft()()

    nc.finalize()
    return nc


def _get_nc(causal: bool):
    key = bool(causal)
    if key not in _BUILD_CACHE:
        _BUILD_CACHE[key] = _build(causal)
    return _BUILD_CACHE[key]


def _rope_tables(position_ids_b):
    # cosT/sinT: [HD, S] fp32, transposed layout for the [d, s] dataflow
    pos = np.asarray(position_ids_b, dtype=np.float64)
    inv = 1.0 / (ROPE_BASE ** (np.arange(0, HD, 2, dtype=np.float64) / HD))
    f = pos[:, None] * inv[None, :]            # [S, HD/2]
    emb = np.concatenate([f, f], axis=1)       # [S, HD]
    cosT = np.ascontiguousarray(np.cos(emb).T.astype(np.float32))
    sinT = np.ascontiguousarray(np.sin(emb).T.astype(np.float32))
    return cosT, sinT


def _is_causal(attention_mask):
    m = np.asarray(attention_mask)
    if m.shape != (B, 1, S, S):
        return False
    tri = np.tril(np.ones((S, S), dtype=bool))
    canon = np.where(tri, np.float32(0.0), np.float32(-1e9))
    return all(np.array_equal(m[b, 0], canon) for b in range(B))


_ONES_NP = np.ones((P, P), dtype=np.float32)
_ONES_BF = np.ones((P, P), dtype=ml_dtypes.bfloat16)
_IDENT_BF = np.eye(P, dtype=np.float32).astype(ml_dtypes.bfloat16)


NEG_MASK = -1.0e4  # additive logit mask; exp(-1e4/16) == 0 exactly in fp32


def _stair():
    # additive staircase: 0 where (j - 512) >= p (keep), -1e4 above diagonal
    j = np.arange(2 * SBLK)[None, :] - SBLK
    p = np.arange(P)[:, None]
    return np.where(j >= p, 0.0, NEG_MASK).astype(ml_dtypes.bfloat16)


def _bf(x):
    return np.ascontiguousarray(np.asarray(x, dtype=np.float32).astype(ml_dtypes.bfloat16))


def kernel(hidden_state, attention_mask, position_ids, Wq, Wk, Wv, Wo,
           _trace=False, _tmpdir=None):
    global LAST_EXEC_TIME_NS
    hidden_state = np.asarray(hidden_state, dtype=np.float32)

    causal = _is_causal(attention_mask)
    nc = _get_nc(causal)

    stair = _stair() if causal else None
    wk_bf = _bf(Wk)
    wv_bf = _bf(Wv)
    per_batch = {}
    for b in range(B):
        hTb = _bf(hidden_state[b].T)                           # [D, S] bf16
        cosT, sinT = _rope_tables(position_ids[b])
        mb = None
        if not causal:
            # additive mask in pre-scale logit units: exp((pss + m)/16)
            m16 = 16.0 * np.asarray(attention_mask, dtype=np.float64)[b, 0].T
            mb = np.ascontiguousarray(
                np.maximum(m16, NEG_MASK).astype(ml_dtypes.bfloat16))
        per_batch[b] = (hTb, cosT, sinT, mb)

    in_maps = []
    for core in range(8):
        b = core // 4
        hp = core % 4
        hTb, cosT, sinT, mb = per_batch[b]
        im = {
            "hT": hTb,
            "ones": _ONES_NP,
            "onesb": _ONES_BF,
            "identb": _IDENT_BF,
            "wq": _bf(Wq[:, hp * DQ:(hp + 1) * DQ]),
            "wk": wk_bf,
            "wv": wv_bf,
            "wo": _bf(Wo[hp * DQ:(hp + 1) * DQ, :]),
            "cosT": cosT,
            "sinT": sinT,
        }
        if causal:
            im["stair"] = stair
        else:
            im["amaskT"] = mb
        in_maps.append(im)

    res = run_bass_kernel_spmd(nc, in_maps, core_ids=list(range(8)),
                               trace=_trace, tmpdir=_tmpdir)
    LAST_EXEC_TIME_NS = res.exec_time_ns

    out = np.empty((B, S, D), dtype=np.float32)
    for b in range(B):
        acc = res.results[4 * b]["out_partial"].astype(np.float32).copy()
        for hp in range(1, 4):
            acc += res.results[4 * b + hp]["out_partial"]
        out[b] = acc
    return out


# revision 42
# speedup vs baseline: 1.1218x; 1.1022x over previous
"""GemmaAttention (B=2, S=2048, D=2048, H=8, KV=1, HD=256) on 8 trn2 NeuronCores.

Sharding: DP=2 over batch x TP=4 over head-pairs. Core c handles batch c//4 and
heads {2*(c%4), 2*(c%4)+1}. Each core computes its partial o_proj output
(row-parallel Wo); the host sums the 4 partials per batch (the all-reduce is
folded into the host-side unshard).

All PE matmuls run in bf16 (error budget allows it): LDWEIGHTS is 1 cycle/row
instead of fp32r's ~4, and PE power/p-state behaves better. PSUM tiles are
always a full bank, so the kernel is structured around at most 8 live psum
tiles with single-group accumulation chains to keep the PE gap-free (the PE
only reaches its 2.4 GHz p-state after ~3us of continuous execution).

Dataflow per core:
  phase A (projections, full hT resident in SBUF as bf16):
    per s-block of 512 and output pair: psum <- sum_c W[:,c,pair].T @ hT[c,blk]
    RoPE applied in the psum->SBUF drain on DVE, output bf16 (QT/KT).
    V computed directly in [s, dv] layout: psum <- hT_chunk.T @ Wv, drained on
    the scalar engine to bf16 (VN).
  phase C (attention + interleaved o_proj):
    scoresT[k,q] = KT_chunk.T @ QT per head, exp on ACT (scale 1/16 folded),
    causal staircase as a bf16 multiplicative mask, denominators accumulated
    on DVE; outT[dv,q] += V_chunk.T @ expT; normalize with
    reciprocal_approx_fast; out_partial = outTn_chunk.T @ Wo -> DRAM.
"""

import numpy as np
import ml_dtypes

import concourse.bass as bass
import concourse.tile as tile
import concourse.mybir as mybir
from concourse import bacc
from concourse.bass_utils import run_bass_kernel_spmd
from concourse._compat import with_exitstack  # noqa: F401

P = 128
B, S, D = 2, 2048, 2048
H, KV, HD = 8, 1, 256
ROPE_BASE = 10000.0

HEADS_PER_CORE = 2
DQ = HEADS_PER_CORE * HD          # 512 q-dims per core
DCH = D // P                      # 16 contraction chunks
SBLK = 512                        # s-tile for projection rhs / q-tile
NSBLK = S // SBLK                 # 4
NKC = S // P                      # 16 key chunks
NQCH = DQ // P                    # 4 QT partition chunks
NKCH = HD // P                    # 2 KT partition chunks

F32 = mybir.dt.float32
F32R = mybir.dt.float32r
BF16 = mybir.dt.bfloat16
EXP = mybir.ActivationFunctionType.Exp

# exec time of the last traced run (set by run_spmd when tracing)
LAST_EXEC_TIME_NS = None

_BUILD_CACHE = {}


def _build(causal: bool):
    nc = bacc.Bacc()

    hT = nc.declare_dram_parameter("hT", [D, S], BF16, isOutput=False)
    wq = nc.declare_dram_parameter("wq", [D, DQ], BF16, isOutput=False)
    wk = nc.declare_dram_parameter("wk", [D, HD], BF16, isOutput=False)
    wv = nc.declare_dram_parameter("wv", [D, HD], BF16, isOutput=False)
    wo = nc.declare_dram_parameter("wo", [DQ, D], BF16, isOutput=False)
    cosT = nc.declare_dram_parameter("cosT", [HD, S], F32, isOutput=False)
    sinT = nc.declare_dram_parameter("sinT", [HD, S], F32, isOutput=False)
    ones = nc.declare_dram_parameter("ones", [P, P], F32R, isOutput=False)
    onesb = nc.declare_dram_parameter("onesb", [P, P], BF16, isOutput=False)
    identb = nc.declare_dram_parameter("identb", [P, P], BF16, isOutput=False)
    if causal:
        stair = nc.declare_dram_parameter("stair", [P, 2 * SBLK], BF16, isOutput=False)
    else:
        maskT = nc.declare_dram_parameter("amaskT", [S, S], BF16, isOutput=False)
    outp = nc.declare_dram_parameter("out_partial", [S, D], BF16, isOutput=True)

    from contextlib import ExitStack
    from collections import deque
    with tile.TileContext(nc) as tc, ExitStack() as ctx:
        # persistent pools
        pq = ctx.enter_context(tc.tile_pool(name="pq", bufs=1))
        QT = pq.tile([P, NQCH, S], BF16, name="QT")
        KT = pq.tile([P, NKCH, S], BF16, name="KT")
        VN = pq.tile([P, NKC, HD], BF16, name="VN")
        ONES = pq.tile([P, P], F32R, name="ONES")
        ONESB = pq.tile([P, P], BF16, name="ONESB")
        IDENTB = pq.tile([P, P], BF16, name="IDENTB")
        ONEC = ONES[:, 0:1]
        ONERB = ONESB[0:1, :]
        if causal:
            STAIR = pq.tile([P, 2 * SBLK], BF16, name="STAIR")

        # ---- phase A: projections + RoPE --------------------------------
        with tc.tile_pool(name="pht", bufs=1) as pht, \
             tc.tile_pool(name="pw", bufs=1) as pw, \
             tc.tile_pool(name="pcs", bufs=1) as pcs, \
             tc.tile_pool(name="ptmp", bufs=8) as ptmp, \
             tc.tile_pool(name="pjp", bufs=8, space="PSUM") as pp:
            HT = pht.tile([P, DCH, S], BF16, name="HT")
            WQ = pw.tile([P, DCH, DQ], BF16, name="WQ")
            WK = pw.tile([P, DCH, HD], BF16, name="WK")
            WV = pw.tile([P, DCH, HD], BF16, name="WV")
            COS = pcs.tile([P, NKCH, S], F32, name="COS")
            SIN = pcs.tile([P, NKCH, S], F32, name="SIN")

            # DMA order matches consumption. Multi-chunk 3D transfers keep the
            # sync queue from becoming issue-bound (~620ns per descriptor).
            nc.sync.dma_start(out=ONES, in_=ones[:, :])
            nc.sync.dma_start(out=ONESB, in_=onesb[:, :])
            nc.sync.dma_start(out=IDENTB, in_=identb[:, :])
            if causal:
                nc.sync.dma_start(out=STAIR, in_=stair[:, :])

            def chunked3d(out_tile, dram, c0, c1, csl):
                # out_tile[:, c0:c1, csl] <- dram rows [c0*P, c1*P), cols csl
                nc.sync.dma_start(
                    out=out_tile[:, c0:c1, csl],
                    in_=dram.rearrange("(c p) n -> p c n", p=P)[:, c0:c1, csl])

            full = slice(0, None)
            # sb0: WQ and hT slabs interleaved, small first slab for fast start
            for c0, c1 in ((0, 2), (2, 6), (6, 11), (11, 16)):
                chunked3d(WQ, wq, c0, c1, full)
                chunked3d(HT, hT, c0, c1, slice(0, SBLK))
            chunked3d(COS, cosT, 0, NKCH, slice(0, SBLK))
            chunked3d(SIN, sinT, 0, NKCH, slice(0, SBLK))
            chunked3d(WK, wk, 0, DCH, full)
            chunked3d(WV, wv, 0, DCH, full)
            for sb in range(1, NSBLK):
                ssl = slice(sb * SBLK, (sb + 1) * SBLK)
                chunked3d(HT, hT, 0, DCH, ssl)
                chunked3d(COS, cosT, 0, NKCH, ssl)
                chunked3d(SIN, sinT, 0, NKCH, ssl)

            def rope_pair(p0, p1, out0, out1):
                # out0 = p0*cos0 - p1*sin0 ; out1 = p1*cos1 + p0*sin1
                c0 = COS[:, 0, ssl]; c1 = COS[:, 1, ssl]
                s0 = SIN[:, 0, ssl]; s1 = SIN[:, 1, ssl]
                t1 = ptmp.tile([P, SBLK], F32, name="t")
                t2 = ptmp.tile([P, SBLK], F32, name="t")
                nc.vector.tensor_mul(t1, p0, c0)
                nc.vector.tensor_mul(t2, p1, s0)
                nc.vector.tensor_sub(out0, t1, t2)
                t3 = ptmp.tile([P, SBLK], F32, name="t")
                t4 = ptmp.tile([P, SBLK], F32, name="t")
                nc.vector.tensor_mul(t3, p1, c1)
                nc.vector.tensor_mul(t4, p0, s1)
                nc.vector.tensor_add(out1, t3, t4)

            for sb in range(NSBLK):
                ssl = slice(sb * SBLK, (sb + 1) * SBLK)
                # Q pairs (per head), then K pair: 16-matmul psum groups
                for h in range(HEADS_PER_CORE):
                    p0 = pp.tile([P, SBLK], F32, name="pp")
                    p1 = pp.tile([P, SBLK], F32, name="pp")
                    for c in range(DCH):
                        nc.tensor.matmul(p0, lhsT=WQ[:, c, 2 * h * P:(2 * h + 1) * P],
                                         rhs=HT[:, c, ssl],
                                         start=(c == 0), stop=(c == DCH - 1))
                        nc.tensor.matmul(p1, lhsT=WQ[:, c, (2 * h + 1) * P:(2 * h + 2) * P],
                                         rhs=HT[:, c, ssl],
                                         start=(c == 0), stop=(c == DCH - 1))
                    rope_pair(p0, p1, QT[:, 2 * h, ssl], QT[:, 2 * h + 1, ssl])
                p0 = pp.tile([P, SBLK], F32, name="pp")
                p1 = pp.tile([P, SBLK], F32, name="pp")
                for c in range(DCH):
                    nc.tensor.matmul(p0, lhsT=WK[:, c, 0:P], rhs=HT[:, c, ssl],
                                     start=(c == 0), stop=(c == DCH - 1))
                    nc.tensor.matmul(p1, lhsT=WK[:, c, P:2 * P], rhs=HT[:, c, ssl],
                                     start=(c == 0), stop=(c == DCH - 1))
                rope_pair(p0, p1, KT[:, 0, ssl], KT[:, 1, ssl])
                # V directly in [s, dv] layout (lhsT = hT chunk slice)
                for si in range(SBLK // P):
                    sv = pp.tile([P, SBLK], F32, name="pp")[:, 0:HD]
                    soff = sb * SBLK + si * P
                    for c in range(DCH):
                        nc.tensor.matmul(sv, lhsT=HT[:, c, soff:soff + P],
                                         rhs=WV[:, c, :],
                                         start=(c == 0), stop=(c == DCH - 1))
                    nc.scalar.copy(VN[:, sb * (SBLK // P) + si, :], sv)

        # ---- late persistent: o_proj weights + normalized outT ----------
        patt = ctx.enter_context(tc.tile_pool(name="patt", bufs=1))
        WO = patt.tile([P, NQCH, D], BF16, name="WO")
        nc.sync.dma_start(out=WO, in_=wo.rearrange("(c p) n -> p c n", p=P))
        OUTN = patt.tile([P, NQCH, S], BF16, name="OUTN")

        # ---- phase C: attention (both heads interleaved) + o_proj -------
        # One kc loop drives both heads: the PE gets 2x the independent work
        # between a score tile and its AV consumer, so the exp->mask chain has
        # ~2us of slack. Norm and o_proj work from the previous q-block is
        # injected between kc steps as small jobs (never bursts), keeping the
        # ACT/DVE queues shallow for the latency-critical exp path.
        with tc.tile_pool(name="pexp", bufs=10) as pexp, \
             tc.tile_pool(name="pacc", bufs=12) as pacc, \
             tc.tile_pool(name="pou", bufs=8) as pou, \
             tc.tile_pool(name="pmisc", bufs=8) as pmisc, \
             tc.tile_pool(name="pmask", bufs=2) as pmask, \
             tc.tile_pool(name="pfin", bufs=4) as pfin, \
             tc.tile_pool(name="ps_s", bufs=3, space="PSUM") as ps_s, \
             tc.tile_pool(name="ps_o", bufs=4, space="PSUM") as ps_o, \
             tc.tile_pool(name="ps_f", bufs=1, space="PSUM") as ps_f:

            def make_norm_jobs(ous, accs, qb):
                qsl = slice(qb * SBLK, (qb + 1) * SBLK)
                rsbbs = {}

                def ja(h):
                    def f():
                        pssum = ps_s.tile([P, SBLK], F32, name="ps")
                        nc.tensor.matmul(pssum[0:1, :], lhsT=ONEC, rhs=accs[h])
                        rsb = pmisc.tile([1, SBLK], F32, name="rsb")
                        rsbb = pmisc.tile([1, SBLK], BF16, name="rsbb")
                        with nc.allow_low_precision("approx softmax recip"):
                            nc.vector.reciprocal_approx_fast(out=rsb,
                                                             in_=pssum[0:1, :])
                            nc.vector.tensor_copy(rsbb, rsb)
                        rsbbs[h] = rsbb
                    return f

                def jb(h):
                    def f():
                        psb = ps_s.tile([P, SBLK], F32, name="ps")
                        nc.tensor.matmul(psb, lhsT=ONERB, rhs=rsbbs[h])
                        rbc = pmisc.tile([P, SBLK], F32R, name="rbc")
                        nc.scalar.copy(rbc, psb)
                        for dvc in range(2):
                            nc.vector.tensor_mul(OUTN[:, 2 * h + dvc, qsl],
                                                 ous[h][dvc], rbc)
                    return f

                return [ja(0), ja(1), jb(0), jb(1)]

            def make_oproj_jobs(qb):
                # one job per (st, nb) output tile: 4 matmuls + drain + DMA
                jobs = []
                for st in range(4 * qb, 4 * qb + 4):
                    for nb in range(NSBLK):
                        def f(st=st, nb=nb, pool=None):
                            stsl = slice(st * P, (st + 1) * P)
                            psf = (ps_f.tile([P, SBLK], F32, name="pf")
                                   if pool is None else
                                   pool.tile([P, SBLK], F32, name="po"))
                            for dvc in range(NQCH):
                                nc.tensor.matmul(
                                    psf, lhsT=OUTN[:, dvc, stsl],
                                    rhs=WO[:, dvc, nb * SBLK:(nb + 1) * SBLK],
                                    start=(dvc == 0), stop=(dvc == NQCH - 1))
                            # bf16 drain (halves output DMA); mostly on DVE,
                            # every 4th on ACT (exp keeps ACT ~80% busy)
                            fsb = pfin.tile([P, SBLK], BF16, name="fsb")
                            if nb % 4 == 0:
                                nc.scalar.copy(fsb, psf)
                            else:
                                with nc.allow_low_precision("bf16 o_proj out"):
                                    nc.vector.tensor_copy(fsb, psf)
                            nc.sync.dma_start(
                                out=outp[stsl, nb * SBLK:(nb + 1) * SBLK],
                                in_=fsb)
                        jobs.append(f)
                return jobs

            backlog = deque()
            for qb in range(NSBLK):
                qsl = slice(qb * SBLK, (qb + 1) * SBLK)
                klim = 4 * (qb + 1) if causal else NKC
                MT = None
                if not causal:
                    MT = pmask.tile([P, NKC, SBLK], BF16, name="mt")
                    nc.sync.dma_start(
                        out=MT,
                        in_=maskT.rearrange("(c p) n -> p c n", p=P)[:, :, qsl])

                pso = {h: [ps_o.tile([P, SBLK], F32, name="po")
                           for _ in range(2)] for h in range(2)}
                exs = {}
                parts = {0: [], 1: []}

                # pairwise denominator tree: leaf merges on gpsimd (idle),
                # higher ranks on DVE; chain depth after the last ex is ~2
                def acc_push(h, node):
                    rank = 0
                    while parts[h] and parts[h][-1][0] == rank:
                        _, prev = parts[h].pop()
                        eng = nc.gpsimd if rank == 0 else nc.vector
                        t = pacc.tile([P, SBLK], F32R, name="acc")
                        with nc.allow_low_precision("softmax denom partial"):
                            eng.tensor_add(t, prev, node)
                        node = t
                        rank += 1
                    parts[h].append((rank, node))

                def acc_flush(h):
                    _, node = parts[h].pop()
                    while parts[h]:
                        _, prev = parts[h].pop()
                        t = pacc.tile([P, SBLK], F32R, name="acc")
                        nc.vector.tensor_add(t, prev, node)
                        node = t
                    return node

                def emit_scores(h, kc):
                    # causal/external mask folded into the PE as a third
                    # matmul (identity lhsT x additive -1e4 staircase): no
                    # DVE op in the exp->AV latency chain, exp(-625) == 0
                    mask_rhs = None
                    if causal and kc >= 4 * qb:
                        delta = 128 * kc - 512 * qb
                        mask_rhs = STAIR[:, 512 - delta:1024 - delta]
                    elif not causal:
                        mask_rhs = MT[:, kc, :]
                    pss = ps_s.tile([P, SBLK], F32, name="ps")
                    for c in range(NKCH):
                        nc.tensor.matmul(pss,
                                         lhsT=KT[:, c, kc * P:(kc + 1) * P],
                                         rhs=QT[:, 2 * h + c, qsl],
                                         start=(c == 0),
                                         stop=(c == NKCH - 1 and mask_rhs is None))
                    if mask_rhs is not None:
                        nc.tensor.matmul(pss, lhsT=IDENTB, rhs=mask_rhs,
                                         start=False, stop=True)
                    ex = pexp.tile([P, SBLK], BF16, name="ex")
                    nc.scalar.activation(ex, pss, EXP, scale=1.0 / 16.0)
                    acc_push(h, ex)
                    exs[(h, kc)] = ex

                def emit_av(h, kc):
                    ex = exs.pop((h, kc))
                    for dvc in range(2):
                        nc.tensor.matmul(pso[h][dvc],
                                         lhsT=VN[:, kc, dvc * P:(dvc + 1) * P],
                                         rhs=ex, start=(kc == 0),
                                         stop=(kc == klim - 1))

                bl = list(backlog)
                norm_jobs, st_jobs = (bl[:4], bl[4:]) if len(bl) >= 4 else ([], bl)
                backlog = deque()
                nst, ptr = len(st_jobs), 0
                npts = max(1, klim - 7)
                for kc in range(klim):
                    emit_scores(0, kc)
                    emit_scores(1, kc)
                    # norm jobs pinned early: ja pair at kc=2, jb pair at kc=4
                    # (the jb matmul then has ~2 steps of slack on the recip);
                    # o_proj jobs spread over the remaining steps
                    if kc == 2 and norm_jobs:
                        norm_jobs[0](); norm_jobs[1]()
                    if kc == 6 and norm_jobs:
                        norm_jobs[2](); norm_jobs[3]()
                    if kc >= 7:
                        want = nst * (kc - 6) // npts
                        while ptr < want:
                            st_jobs[ptr]()
                            ptr += 1
                    if kc >= 2:
                        emit_av(0, kc - 2)
                        emit_av(1, kc - 2)
                while ptr < nst:
                    st_jobs[ptr]()
                    ptr += 1
                for kc in range(max(0, klim - 2), klim):
                    emit_av(0, kc)
                    emit_av(1, kc)
                accs = {h: acc_flush(h) for h in range(2)}
                ous = {}
                for h in range(2):
                    ous[h] = [pou.tile([P, SBLK], BF16, name="ou")
                              for _ in range(2)]
                    for dvc in range(2):
                        # drain AV psums on ACT: frees the pso banks fast and
                        # keeps the DVE queue short for the upcoming recip
                        nc.scalar.copy(ous[h][dvc], pso[h][dvc])
                backlog.extend(make_norm_jobs(ous, accs, qb))
                backlog.extend(make_oproj_jobs(qb))
            # final drain: attention is over, so the o_proj tail can pipeline
            # through the freed 4-bank AV psum pool instead of ps_f's single
            for idx, job in enumerate(backlog):
                if idx >= 4:
                    job(pool=ps_o)
                else:
                    job()

    nc.finalize()
    return nc


def _get_nc(causal: bool):
    key = bool(causal)
    if key not in _BUILD_CACHE:
        _BUILD_CACHE[key] = _build(causal)
    return _BUILD_CACHE[key]


def _rope_tables(position_ids_b):
    # cosT/sinT: [HD, S] fp32, transposed layout for the [d, s] dataflow
    pos = np.asarray(position_ids_b, dtype=np.float64)
    inv = 1.0 / (ROPE_BASE ** (np.arange(0, HD, 2, dtype=np.float64) / HD))
    f = pos[:, None] * inv[None, :]            # [S, HD/2]
    emb = np.concatenate([f, f], axis=1)       # [S, HD]
    cosT = np.ascontiguousarray(np.cos(emb).T.astype(np.float32))
    sinT = np.ascontiguousarray(np.sin(emb).T.astype(np.float32))
    return cosT, sinT


def _is_causal(attention_mask):
    m = np.asarray(attention_mask)
    if m.shape != (B, 1, S, S):
        return False
    tri = np.tril(np.ones((S, S), dtype=bool))
    canon = np.where(tri, np.float32(0.0), np.float32(-1e9))
    return all(np.array_equal(m[b, 0], canon) for b in range(B))


_ONES_NP = np.ones((P, P), dtype=np.float32)
_ONES_BF = np.ones((P, P), dtype=ml_dtypes.bfloat16)
_IDENT_BF = np.eye(P, dtype=np.float32).astype(ml_dtypes.bfloat16)


NEG_MASK = -1.0e4  # additive logit mask; exp(-1e4/16) == 0 exactly in fp32


def _stair():
    # additive staircase: 0 where (j - 512) >= p (keep), -1e4 above diagonal
    j = np.arange(2 * SBLK)[None, :] - SBLK
    p = np.arange(P)[:, None]
    return np.where(j >= p, 0.0, NEG_MASK).astype(ml_dtypes.bfloat16)


def _bf(x):
    return np.ascontiguousarray(np.asarray(x, dtype=np.float32).astype(ml_dtypes.bfloat16))


def kernel(hidden_state, attention_mask, position_ids, Wq, Wk, Wv, Wo,
           _trace=False, _tmpdir=None):
    global LAST_EXEC_TIME_NS
    hidden_state = np.asarray(hidden_state, dtype=np.float32)

    causal = _is_causal(attention_mask)
    nc = _get_nc(causal)

    stair = _stair() if causal else None
    wk_bf = _bf(Wk)
    wv_bf = _bf(Wv)
    per_batch = {}
    for b in range(B):
        hTb = _bf(hidden_state[b].T)                           # [D, S] bf16
        cosT, sinT = _rope_tables(position_ids[b])
        mb = None
        if not causal:
            # additive mask in pre-scale logit units: exp((pss + m)/16)
            m16 = 16.0 * np.asarray(attention_mask, dtype=np.float64)[b, 0].T
            mb = np.ascontiguousarray(
                np.maximum(m16, NEG_MASK).astype(ml_dtypes.bfloat16))
        per_batch[b] = (hTb, cosT, sinT, mb)

    in_maps = []
    for core in range(8):
        b = core // 4
        hp = core % 4
        hTb, cosT, sinT, mb = per_batch[b]
        im = {
            "hT": hTb,
            "ones": _ONES_NP,
            "onesb": _ONES_BF,
            "identb": _IDENT_BF,
            "wq": _bf(Wq[:, hp * DQ:(hp + 1) * DQ]),
            "wk": wk_bf,
            "wv": wv_bf,
            "wo": _bf(Wo[hp * DQ:(hp + 1) * DQ, :]),
            "cosT": cosT,
            "sinT": sinT,
        }
        if causal:
            im["stair"] = stair
        else:
            im["amaskT"] = mb
        in_maps.append(im)

    res = run_bass_kernel_spmd(nc, in_maps, core_ids=list(range(8)),
                               trace=_trace, tmpdir=_tmpdir)
    LAST_EXEC_TIME_NS = res.exec_time_ns

    out = np.empty((B, S, D), dtype=np.float32)
    for b in range(B):
        acc = res.results[4 * b]["out_partial"].astype(np.float32).copy()
        for hp in range(1, 4):
            acc += res.results[4 * b + hp]["out_partial"]
        out[b] = acc
    return out
